# revision 4
# baseline (speedup 1.0000x reference)
"""Trainium2 Bass kernel for the 3-layer dual-head GAT (nn_DualHeadGAT), v2.

Strategy (dst-range sharded edges, bf16 tables, no layer-0 AllGather):
  - Nodes split contiguously across 8 cores (6250 each); edges sorted by
    destination so all segment reductions are core-local.
  - Per-node feature tables are bf16 rows [h | es | ed | pad]:
    layers 0/1: 384 cols (768B rows), layer 2: 128 cols (256B rows).
  - Layer 0's table is computed IN FULL on every core from x (x is tiny),
    laid out *rotated* so each core's own nodes come first; this removes
    the layer-0 AllGather. Layers 1/2 AllGather bf16 tables.
  - Edges are processed in superblocks of SB dst-blocks: one dma_gather
    per (lo, hi, ed) region per superblock (Pool-engine SWDGE call
    overhead is ~1us each), and all per-edge elementwise math runs as one
    instruction per superblock.  Per edge, the src row (h+es) is gathered
    by rotated/global src id (int16 split at 32768); the dst row slice
    (ed) is gathered from the core-local table with local dst ids.
  - Scatter-add per 128-dst-node block via one-hot matmul on the PE:
      U = sum_j M_j^T @ [ea*h | ea],  M[e, n] = (dst_local[e] == n)
    with M and rhs in bf16 (PSUM accumulates f32). Self-loops are folded
    in as one extra identity-matrix slot per block.
  - Per-node normalize: out = relu(U[:, :OC]/U[:, OC:] + b); next-layer
    rows h2 = out @ W' via PE transpose + f32r matmul (full precision
    weights), written bf16.

Self-contained: hardcodes the problem shapes; host-side preprocessing of
edge_index is pure index manipulation. All float math runs on device.
"""
import math
from contextlib import ExitStack
import numpy as np
import ml_dtypes

import concourse.bass as bass
import concourse.bacc as bacc
import concourse.mybir as mybir
import concourse.tile as tile
from concourse.bass_utils import run_bass_kernel_spmd
from concourse.tile_rust import add_dep_helper

F32 = mybir.dt.float32
F32R = mybir.dt.float32r
BF16 = mybir.dt.bfloat16
I16 = mybir.dt.int16
BF_NP = ml_dtypes.bfloat16

P = 128
NEG = 0.2


class Geo:
    def __init__(self, n=50000, ncores=8, split=32768):
        self.N = n
        self.NCORES = ncores
        self.NPD = n // ncores
        self.NBLK = math.ceil(self.NPD / P)
        self.NBLK_ALL = math.ceil(n / P)
        self.SPLIT = split


GEO = Geo()

LAYERS = [(2, 4, 64), (256, 4, 64), (256, 1, 2)]
ROWW = [384, 384, 128]     # bf16 row width of each layer's table
USED = [264, 264, 4]       # h | es | ed used cols
ES0 = [256, 256, 2]        # es col offset within row

SB = 2                     # dst-blocks per gather superblock
CALL_SLOTS = 8            # max 128-edge slots per dma_gather call
SINGLE_PACKET = True      # multi-packet SWDGE for big calls
QROT = 4                   # rotate gather calls over this many SWDGE queues
                           # (sim's sem-lane/queue affinity check requires 1)
STAGE = 0                  # 0=full, 1=phaseA only, 2=+L0, 3=+AG1+L1


# --------------------------------------------------------------------------
# host preprocessing
# --------------------------------------------------------------------------

def _wrap16(seq):
    """dma_gather index layout: idx i lives at [i % 16, i // 16]; tiled to
    128 partitions."""
    seq = np.asarray(seq, np.int16)
    a = seq.reshape(-1, 16).T
    return np.tile(a, (8, 1))


def _build_struct(src_key, dst_all, geo, sbsz):
    """Slot structure for one gather-key function, superblock-ordered.

    Slot layout: per superblock of sbsz dst-blocks:
    [lo(k0) lo(k1) ... hi(k0) hi(k1) ...]."""
    N, NCORES, NPD, NBLK, SPLIT = (geo.N, geo.NCORES, geo.NPD, geo.NBLK,
                                   geo.SPLIT)
    cnt_lo = np.zeros((NCORES, NBLK), np.int64)
    cnt_hi = np.zeros((NCORES, NBLK), np.int64)
    parts = {}
    for d in range(NCORES):
        for k in range(NBLK):
            i = d * NBLK + k
            keys = src_key(d, i)
            dsts = dst_all[i]
            lo = keys < SPLIT
            parts[(d, k)] = (keys[lo], dsts[lo], keys[~lo] - SPLIT, dsts[~lo])
            cnt_lo[d, k] = int(lo.sum())
            cnt_hi[d, k] = int((~lo).sum())

    S_lo = np.ceil(cnt_lo.max(axis=0) / P).astype(np.int64)
    S_hi = np.ceil(cnt_hi.max(axis=0) / P).astype(np.int64)
    olo = np.concatenate([[0], np.cumsum(S_lo)]).astype(np.int64)
    ohi = np.concatenate([[0], np.cumsum(S_hi)]).astype(np.int64)
    T_lo, T_hi = int(olo[-1]), int(ohi[-1])

    blk_lo_pos = np.zeros(NBLK, np.int64)
    blk_hi_pos = np.zeros(NBLK, np.int64)
    sb_bounds = []
    pos = 0
    for k0 in range(0, NBLK, sbsz):
        k1 = min(k0 + sbsz, NBLK)
        Llo = int(S_lo[k0:k1].sum())
        Lhi = int(S_hi[k0:k1].sum())
        pp = pos
        for k in range(k0, k1):
            blk_lo_pos[k] = pp
            pp += S_lo[k]
        for k in range(k0, k1):
            blk_hi_pos[k] = pp
            pp += S_hi[k]
        sb_bounds.append((k0, k1, pos, Llo, Lhi))
        pos += Llo + Lhi
    T = pos

    tabs = []
    for d in range(NCORES):
        idx_lo = np.zeros((P, 8 * max(T_lo, 1)), np.int16)
        idx_hi = np.zeros((P, 8 * max(T_hi, 1)), np.int16)
        idx_ed = np.zeros((P, 8 * max(T, 1)), np.int16)
        dloc = np.full((P, max(T, 1)), 999.0, BF_NP)

        def place(pos0, nslots, keys, dsts, base, tab):
            if nslots == 0:
                return
            buf = np.zeros(nslots * P, np.int16)
            buf[:len(keys)] = keys.astype(np.int16)
            tab[:, 8 * pos0:8 * (pos0 + nslots)] = _wrap16(buf)

        for k in range(NBLK):
            kl, dl, kh, dh = parts[(d, k)]
            slo, shi = int(S_lo[k]), int(S_hi[k])
            base = d * NPD + k * P
            place(int(olo[k]), slo, kl, dl, base, idx_lo)
            place(int(ohi[k]), shi, kh, dh, base, idx_hi)

            for pos0, nsl, dd in ((int(blk_lo_pos[k]), slo, dl),
                                  (int(blk_hi_pos[k]), shi, dh)):
                if nsl == 0:
                    continue
                buf = np.zeros(nsl * P, np.int16)
                buf[:len(dd)] = (dd - d * NPD).astype(np.int16)
                idx_ed[:, 8 * pos0:8 * (pos0 + nsl)] = _wrap16(buf)
                loc = np.full(nsl * P, 999.0, np.float32)
                loc[:len(dd)] = (dd - base).astype(np.float32)
                dloc[:, pos0:pos0 + nsl] = \
                    loc.reshape(nsl, P).T.astype(BF_NP)
        tabs.append((idx_lo, idx_hi, idx_ed, dloc))

    plan = {
        "S_lo": tuple(int(s) for s in S_lo),
        "S_hi": tuple(int(s) for s in S_hi),
        "olo": tuple(int(o) for o in olo),
        "ohi": tuple(int(o) for o in ohi),
        "blk_lo": tuple(int(v) for v in blk_lo_pos),
        "blk_hi": tuple(int(v) for v in blk_hi_pos),
        "sb": tuple(sb_bounds),
        "T": T, "T_lo": T_lo, "T_hi": T_hi,
    }
    return plan, tabs


def _host_prep(x, edge_index, weights, geo=GEO):
    N, NCORES, NPD, NBLK = geo.N, geo.NCORES, geo.NPD, geo.NBLK
    src = np.asarray(edge_index[0]).astype(np.int64)
    dst = np.asarray(edge_index[1]).astype(np.int64)
    perm = np.argsort(dst, kind="stable")
    s_src = src[perm].astype(np.int64)
    s_dst = dst[perm].astype(np.int64)

    starts, stops = [], []
    for d in range(NCORES):
        for k in range(NBLK):
            starts.append(d * NPD + k * P)
            stops.append(min(d * NPD + (k + 1) * P, (d + 1) * NPD))
    e_lo = np.searchsorted(s_dst, starts)
    e_hi = np.searchsorted(s_dst, stops)
    src_blk = {i: s_src[e_lo[i]:e_hi[i]] for i in range(NCORES * NBLK)}
    dst_blk = {i: s_dst[e_lo[i]:e_hi[i]] for i in range(NCORES * NBLK)}

    # structure A: layer-0 rotated keys; structure B: global keys
    planA, tabsA = _build_struct(
        lambda d, i: (src_blk[i] - d * NPD) % N, dst_blk, geo, SB)
    planB, tabsB = _build_struct(
        lambda d, i: src_blk[i], dst_blk, geo, SB)

    x = np.asarray(x, np.float32)
    in_maps = []
    for d in range(NCORES):
        ilA, ihA, ieA, dlA = tabsA[d]
        ilB, ihB, ieB, dlB = tabsB[d]
        rot = np.roll(np.arange(N), -d * NPD)
        m = {
            "xTr": np.ascontiguousarray(x[rot].T),
            "idx_loA": ilA, "idx_hiA": ihA, "idx_edA": ieA, "dlocA": dlA,
            "idx_loB": ilB, "idx_hiB": ihB, "idx_edB": ieB, "dlocB": dlB,
            "iota_bf": np.tile(np.arange(P, dtype=BF_NP)[None, :], (P, 1)),
            "ident_bf": np.eye(P, dtype=BF_NP),
            "ident_f": np.eye(P, dtype=np.float32),
        }
        for li, (W, a_s, a_d, b) in enumerate(weights):
            fin, H, O = LAYERS[li]
            W = np.asarray(W, np.float32)
            a_s = np.asarray(a_s, np.float32)
            a_d = np.asarray(a_d, np.float32)
            b = np.asarray(b, np.float32)
            As = np.zeros((H * O, H), np.float32)
            Ad = np.zeros((H * O, H), np.float32)
            for h in range(H):
                As[h * O:(h + 1) * O, h] = a_s[h]
                Ad[h * O:(h + 1) * O, h] = a_d[h]
            m[f"W{li}"] = W
            m[f"WT{li}"] = np.ascontiguousarray(W.T)
            m[f"As{li}"] = As
            m[f"Ad{li}"] = Ad
            m[f"b{li}"] = np.tile(b[None, :], (P, 1))
        in_maps.append(m)

    return in_maps, {"A": planA, "B": planB}


# --------------------------------------------------------------------------
# device program
# --------------------------------------------------------------------------

def build_program(plans, geo=GEO, repeat=1):
    N, NCORES, NPD = geo.N, geo.NCORES, geo.NPD
    nc = bacc.Bacc("TRN2", target_bir_lowering=False, debug=False,
                   num_devices=NCORES, num_swdge_queues=4)

    t_in = {}

    def inp(name, shape, dt=F32):
        t_in[name] = nc.dram_tensor(name, shape, dt, kind="ExternalInput").ap()

    pA, pB = plans["A"], plans["B"]
    inp("xTr", [2, N])
    inp("idx_loA", [P, 8 * max(pA["T_lo"], 1)], I16)
    inp("idx_hiA", [P, 8 * max(pA["T_hi"], 1)], I16)
    inp("idx_edA", [P, 8 * max(pA["T"], 1)], I16)
    inp("dlocA", [P, max(pA["T"], 1)], BF16)
    inp("idx_loB", [P, 8 * max(pB["T_lo"], 1)], I16)
    inp("idx_hiB", [P, 8 * max(pB["T_hi"], 1)], I16)
    inp("idx_edB", [P, 8 * max(pB["T"], 1)], I16)
    inp("dlocB", [P, max(pB["T"], 1)], BF16)
    inp("iota_bf", [P, P], BF16)
    inp("ident_bf", [P, P], BF16)
    inp("ident_f", [P, P])
    for li, (fin, H, O) in enumerate(LAYERS):
        OC = H * O
        inp(f"W{li}", [fin, OC])
        inp(f"WT{li}", [OC, fin])
        inp(f"As{li}", [OC, H])
        inp(f"Ad{li}", [OC, H])
        inp(f"b{li}", [P, OC])

    out_own = nc.dram_tensor("out", [NPD, 2], F32, kind="ExternalOutput").ap()

    tbl = [nc.dram_tensor("tbl0", [N, ROWW[0]], BF16, kind="Internal").ap()]
    for li in (1, 2):
        tbl.append(nc.dram_tensor(f"tbl{li}", [N, ROWW[li]], BF16,
                                  kind="Internal", addr_space="Shared").ap())
    howns = [None,
             nc.dram_tensor("hown1", [NPD, ROWW[1]], BF16, kind="Internal").ap(),
             nc.dram_tensor("hown2", [NPD, ROWW[2]], BF16, kind="Internal").ap()]

    with tile.TileContext(nc) as tc:
        _emit(tc, t_in, out_own, tbl, howns, plans, geo, repeat)

    # Post-scheduling: pin each gather's SWDGE queue to its assigned DMASW
    # sem lane (lane rotates per Pool-DMA inst in scheduled order; the sem
    # of lane L must always be updated from one queue, so queue = L % QROT).
    if QROT > 1:
        for blk in nc.m.functions[0].blocks:
            for inst in blk.instructions:
                if isinstance(inst, mybir.InstDMAGatherAnt):
                    proc = getattr(inst, "bass_scheduled_proc", None)
                    if proc is not None and 11 <= proc <= 18:
                        inst.queue_num = (proc - 11) % QROT
    nc.compile()
    return nc


def _emit(tc, t_in, out_own, tbl, howns, plans, geo, repeat):
    nc = tc.nc
    pA, pB = plans["A"], plans["B"]
    Lmax = max(lo + hi for pl in (pA, pB) for (_, _, _, lo, hi) in pl["sb"])

    ctx = ExitStack()
    sb_c = ctx.enter_context(tc.tile_pool(name="const", bufs=1))
    sb_i = ctx.enter_context(tc.tile_pool(name="idx", bufs=1))
    sb = ctx.enter_context(tc.tile_pool(name="work", bufs=2))
    sbg = ctx.enter_context(tc.tile_pool(name="gath", bufs=2))
    sbs = ctx.enter_context(tc.tile_pool(name="stage", bufs=2))
    ps = ctx.enter_context(tc.tile_pool(name="psum", bufs=2, space="PSUM"))
    ps_u = ctx.enter_context(tc.tile_pool(name="psum_u", bufs=2, space="PSUM"))

    # ---- persistent constants ----
    def load_const(name, shape, dt=F32):
        t = sb_c.tile(shape, dt, tag=name)
        nc.sync.dma_start(out=t[:], in_=t_in[name][:])
        return t

    c_iota = load_const("iota_bf", [P, P], BF16)
    c_identb = load_const("ident_bf", [P, P], BF16)
    c_identf = load_const("ident_f", [P, P])
    c_b = [load_const(f"b{li}", [P, LAYERS[li][1] * LAYERS[li][2]])
           for li in range(3)]

    # ---- per-structure gather index tiles (one structure resident) ----
    TLO = max(pA["T_lo"], pB["T_lo"], 1)
    THI = max(pA["T_hi"], pB["T_hi"], 1)
    TT = max(pA["T"], pB["T"], 1)

    def load_struct(sfx):
        p = pA if sfx == "A" else pB
        il = sb_i.tile([P, 8 * TLO], I16, tag="il")
        ih = sb_i.tile([P, 8 * THI], I16, tag="ih")
        ie = sb_i.tile([P, 8 * TT], I16, tag="ie")
        dl = sb_i.tile([P, TT], BF16, tag="dl")
        nc.sync.dma_start(out=il[:, 0:8 * max(p["T_lo"], 1)],
                          in_=t_in[f"idx_lo{sfx}"][:])
        nc.sync.dma_start(out=ih[:, 0:8 * max(p["T_hi"], 1)],
                          in_=t_in[f"idx_hi{sfx}"][:])
        nc.sync.dma_start(out=ie[:, 0:8 * max(p["T"], 1)],
                          in_=t_in[f"idx_ed{sfx}"][:])
        nc.sync.dma_start(out=dl[:, 0:max(p["T"], 1)],
                          in_=t_in[f"dloc{sfx}"][:])
        return il, ih, ie, dl

    # ---- W' = [W | W@As | W@Ad] per layer (f32, tiny) ----
    wprime = []
    for li, (fin, H, O) in enumerate(LAYERS):
        OC = H * O
        n_fin_t = math.ceil(fin / P)
        n_k_t = math.ceil(OC / P)
        kp = min(P, OC)
        tiles = []
        for fi in range(n_fin_t):
            fr = min(P, fin - fi * P)
            wp = sb_c.tile([P, OC + 2 * H], F32R, tag=f"wp{li}_{fi}")
            nc.sync.dma_start(out=wp[:fr, 0:OC],
                              in_=t_in[f"W{li}"][fi * P:fi * P + fr, :]
                              .bitcast(F32R))
            for ci, aname in ((0, f"As{li}"), (1, f"Ad{li}")):
                wa_fl = ps.tile([P, 264], F32, space="PSUM", tag="h0ps")
                wa_ps = wa_fl[:, 0:H]
                a_sb = sb.tile([P, n_k_t, H], F32, tag="a_in")
                nc.sync.dma_start(
                    out=a_sb[:kp, 0:n_k_t, :],
                    in_=t_in[aname][:].rearrange("(a p) h -> p a h", p=kp))
                wt_sb = sb.tile([P, n_k_t, P], F32, tag="wt_in")
                nc.sync.dma_start(
                    out=wt_sb[:kp, 0:n_k_t, 0:fr],
                    in_=t_in[f"WT{li}"][:, fi * P:fi * P + fr].rearrange(
                        "(a p) f -> p a f", p=kp))
                for ki in range(n_k_t):
                    kr = min(P, OC - ki * P)
                    nc.tensor.matmul(
                        out=wa_ps[:fr, :],
                        lhsT=wt_sb[:kr, ki, 0:fr],
                        rhs=a_sb[:kr, ki, :],
                        start=(ki == 0), stop=(ki == n_k_t - 1))
                nc.vector.tensor_copy(
                    out=wp[:fr, OC + ci * H:OC + (ci + 1) * H],
                    in_=wa_ps[:fr, :])
            tiles.append(wp)
        wprime.append(tiles)

    consts = (c_iota, c_identb, c_identf, c_b, wprime, load_struct)
    pools = (sb, sbg, sbs, ps, ps_u)
    for rep in range(repeat):
        _emit_iter(tc, t_in, out_own, tbl, howns, plans, geo, consts, pools,
                   Lmax)

    ctx.close()


def _emit_iter(tc, t_in, out_own, tbl, howns, plans, geo, consts, pools,
               Lmax):
    nc = tc.nc
    N, NCORES, NPD, NBLK, NBLK_ALL, SPLIT = (geo.N, geo.NCORES, geo.NPD,
                                             geo.NBLK, geo.NBLK_ALL, geo.SPLIT)
    (c_iota, c_identb, c_identf, c_b, wprime, load_struct) = consts
    sb, sbg, sbs, ps, ps_u = pools
    pA, pB = plans["A"], plans["B"]

    qctr = [0]

    def next_q():
        return 0

    # ================= phase A: full layer-0 table (rotated) =================
    XCHUNK = 384  # 12 blocks of x columns per SBUF load
    tbl0_writes = []

    def flush_tbl0(stage, blocks):
        full = [(j, gg) for (j, gg, nk) in blocks if nk == P]
        part = [(j, gg, nk) for (j, gg, nk) in blocks if nk != P]
        ws = []
        if full:
            j0, g0 = full[0]
            cnt = len(full)
            w = nc.sync.dma_start(
                out=tbl[0][g0 * P:(g0 + cnt) * P, 0:264].rearrange(
                    "(a p) c -> p a c", p=P),
                in_=stage[:, j0:j0 + cnt, :])
            ws.append(w)
        for (j, gg, nk) in part:
            w = nc.sync.dma_start(
                out=tbl[0][gg * P:gg * P + nk, 0:264],
                in_=stage[:nk, j, :])
            ws.append(w)
        return ws

    B_ST = 4
    g = 0
    eng_rot = [nc.vector, nc.scalar]   # Pool cannot read PSUM
    while g < NBLK_ALL:
        c0 = g * P
        ccols = min(XCHUNK, N - c0)
        nblk_c = math.ceil(ccols / P)
        xc = sb.tile([2, XCHUNK], F32R, tag="xc")
        nc.sync.dma_start(out=xc[:2, 0:ccols],
                          in_=t_in["xTr"][:, c0:c0 + ccols].bitcast(F32R))
        done = 0
        while done < nblk_c:
            grp = min(B_ST, nblk_c - done)
            stage = sbs.tile([P, B_ST, 264], BF16, tag="stA")
            blocks = []
            for j in range(grp):
                gb = g + done + j
                nk = min(P, N - gb * P)
                h0_ps = ps.tile([P, 264], F32, space="PSUM", tag="h0ps")
                nc.tensor.matmul(
                    out=h0_ps[:nk, :],
                    lhsT=xc[:2, (done + j) * P:(done + j) * P + nk],
                    rhs=wprime[0][0][:2, 0:264],
                    start=True, stop=True)
                eng = eng_rot[gb % 2]
                if eng is nc.scalar:
                    nc.scalar.copy(out=stage[:nk, j, :], in_=h0_ps[:nk, :])
                else:
                    eng.tensor_copy(out=stage[:nk, j, :], in_=h0_ps[:nk, :])
                blocks.append((j, gb, nk))
            tbl0_writes += flush_tbl0(stage, blocks)
            done += grp
        g += nblk_c

    bar0 = nc.gpsimd.engine_nop()
    for w in tbl0_writes:
        add_dep_helper(bar0.ins, w.ins, reason="tbl0 barrier")

    if STAGE == 1:
        return

    # ================= layers =================
    h_writes = []
    hw_byblock = {}

    n_layers = {0: 3, 2: 1, 3: 2}[STAGE]
    for li in range(n_layers):
        fin, H, O = LAYERS[li]
        OC = H * O
        RC = OC + H
        roww = ROWW[li]
        es0 = ES0[li]
        # ed col offset inside the gathered 128-col ed row: layers 0/1 gather
        # row cols [256:384] (es at 0, ed at H); layer 2 gathers [0:128]
        edo = (128 + H) if li < 2 else 3
        last = (li == 2)
        pl = pA if li == 0 else pB
        if li == 0:
            struct = load_struct("A")
        elif li == 1:
            struct = load_struct("B")
        c_il, c_ih, c_ie, c_dl = struct
        S_lo, S_hi = pl["S_lo"], pl["S_hi"]
        olo, ohi = pl["olo"], pl["ohi"]
        blk_lo, blk_hi = pl["blk_lo"], pl["blk_hi"]

        if li == 0:
            src_dep = bar0
            ed_src = tbl[0][0:NPD, 128:384]
            self_src = tbl[0]
        else:
            ag = nc.gpsimd.collective_compute(
                "AllGather", mybir.AluOpType.bypass,
                replica_groups=[list(range(NCORES))],
                ins=[howns[li][:]], outs=[tbl[li][:]],
            )
            for w in h_writes:
                add_dep_helper(ag.ins, w.ins, reason="AG after h writes")
            src_dep = ag
            if li == 1:
                ed_src = howns[1][:, 128:384]
            else:
                ed_src = howns[2][:, 0:128]
            self_src = howns[li]
        prev_hw_byblock = hw_byblock
        hw_byblock = {}
        h_writes = []

        def flush_hown(stage, blocks, li2):
            cols2 = USED[li2]
            full = [(j, k) for (j, k, nk) in blocks if nk == P]
            part = [(j, k, nk) for (j, k, nk) in blocks if nk != P]
            ws = []
            if full:
                j0, k0 = full[0]
                cnt = len(full)
                w = nc.sync.dma_start(
                    out=howns[li2][k0 * P:(k0 + cnt) * P, 0:cols2].rearrange(
                        "(a p) c -> p a c", p=P),
                    in_=stage[:, j0:j0 + cnt, 0:cols2])
                ws.append(w)
            for (j, k, nk) in part:
                w = nc.sync.dma_start(
                    out=howns[li2][k * P:k * P + nk, 0:cols2],
                    in_=stage[:nk, j, 0:cols2])
                ws.append(w)
            return ws

        B_H = 4
        hstage = None
        hstage_blocks = []
        B_O = 8
        ostage = None
        ostage_blocks = []

        def _call(out3, o0, in_ap, idxt, ioff, cnt, elem, estep=None,
                  dep=None):
            for cc0 in range(0, cnt, CALL_SLOTS):
                cs = min(CALL_SLOTS, cnt - cc0)
                gi = nc.gpsimd.dma_gather(
                    out_ap=out3[:, o0 + cc0:o0 + cc0 + cs, :],
                    in_ap=in_ap,
                    idxs_ap=idxt[:, 8 * (ioff + cc0):8 * (ioff + cc0 + cs)],
                    num_idxs=cs * P, num_idxs_reg=cs * P,
                    elem_size=elem, elem_step=estep,
                    single_packet=SINGLE_PACKET, queue_num=next_q())
                add_dep_helper(
                    gi.ins, (dep if dep is not None else src_dep).ins,
                    reason="gather after producer")

        for (k0b, k1b, soff, Llo, Lhi) in pl["sb"]:
            Lsb = Llo + Lhi
            nb = k1b - k0b

            g_fl = sbg.tile([P, Lmax * ROWW[0]], BF16, tag="g")
            gt = g_fl[:].rearrange("p (s r) -> p s r", r=roww)
            _call(gt, 0, tbl[li][:], c_il, olo[k0b], Llo, roww)
            _call(gt, Llo, tbl[li][SPLIT:, :], c_ih, ohi[k0b], Lhi, roww)

            e_fl = sbg.tile([P, Lmax * 256], BF16, tag="e")
            et = e_fl[:].rearrange("p (s r) -> p s r",
                                   r=(256 if li < 2 else P))
            if li > 0:
                # ed rows lie in blocks k0b..k1b-1 of the local table
                deps = {id(prev_hw_byblock[k]): prev_hw_byblock[k]
                        for k in range(k0b, k1b) if k in prev_hw_byblock}
                for cc0 in range(0, Lsb, CALL_SLOTS):
                    cs = min(CALL_SLOTS, Lsb - cc0)
                    gi = nc.gpsimd.dma_gather(
                        out_ap=et[:, cc0:cc0 + cs, :], in_ap=ed_src,
                        idxs_ap=c_ie[:, 8 * (soff + cc0):8 * (soff + cc0 + cs)],
                        num_idxs=cs * P, num_idxs_reg=cs * P,
                        elem_size=(256 if li < 2 else P), elem_step=roww,
                        single_packet=SINGLE_PACKET, queue_num=next_q())
                    for dw in deps.values():
                        add_dep_helper(gi.ins, dw.ins,
                                       reason="ed gather after h write")
            else:
                _call(et, 0, ed_src, c_ie, soff, Lsb, 256, roww)

            es_sl = gt[:, 0:Lsb, es0:es0 + H]
            ed_sl = et[:, 0:Lsb, edo:edo + H]
            h_sl = gt[:, 0:Lsb, 0:OC]

            al_fl = sb.tile([P, Lmax * 4], BF16, tag="al")
            al = al_fl[:].rearrange("p (s h) -> p s h", h=H)
            nc.vector.tensor_tensor(out=al[:, 0:Lsb, :], in0=es_sl,
                                    in1=ed_sl, op=mybir.AluOpType.add)
            alk_fl = sb.tile([P, Lmax * 4], BF16, tag="alk")
            alk = alk_fl[:].rearrange("p (s h) -> p s h", h=H)
            nc.vector.scalar_tensor_tensor(
                out=alk[:, 0:Lsb, :], in0=al[:, 0:Lsb, :], scalar=NEG,
                op0=mybir.AluOpType.mult, in1=al[:, 0:Lsb, :],
                op1=mybir.AluOpType.max)
            ea_fl = sb.tile([P, Lmax * 4], BF16, tag="ea")
            ea = ea_fl[:].rearrange("p (s h) -> p s h", h=H)
            nc.scalar.activation(out=ea[:, 0:Lsb, :], in_=alk[:, 0:Lsb, :],
                                 func=mybir.ActivationFunctionType.Exp)

            rhs_fl = sb.tile([P, (Lmax + SB) * 260], BF16, tag="rhs")
            rhs = rhs_fl[:, 0:(Lsb + nb) * RC].rearrange(
                "p (s c) -> p s c", c=RC)
            if Lsb:
                nc.vector.tensor_tensor(
                    out=rhs[:, 0:Lsb, 0:OC].rearrange(
                        "p s (h o) -> p s h o", o=O),
                    in0=h_sl.rearrange("p s (h o) -> p s h o", o=O),
                    in1=ea[:, 0:Lsb, :].unsqueeze(3).to_broadcast(
                        [P, Lsb, H, O]),
                    op=mybir.AluOpType.mult)
                nc.vector.tensor_copy(out=rhs[:, 0:Lsb, OC:RC],
                                      in_=ea[:, 0:Lsb, :])

            m = sb.tile([P, Lmax * P], BF16, tag="m")
            mv = m[:].rearrange("p (s n) -> p s n", n=P)
            if Lsb:
                nc.vector.tensor_tensor(
                    out=mv[:, 0:Lsb, :],
                    in0=c_dl[:, soff:soff + Lsb].unsqueeze(2).to_broadcast(
                        [P, Lsb, P]),
                    in1=c_iota[:].unsqueeze(1).to_broadcast([P, Lsb, P]),
                    op=mybir.AluOpType.is_equal)

            for b in range(nb):
                k = k0b + b
                nk = min(P, NPD - k * P)
                slo, shi = S_lo[k], S_hi[k]

                hb = sb.tile([P, USED[0]], BF16, tag="hb")
                if nk < P:
                    nc.vector.memset(hb[:, 0:USED[li]], 0.0)
                wsl = nc.sync.dma_start(
                    out=hb[:nk, 0:USED[li]],
                    in_=self_src[k * P:k * P + nk, 0:USED[li]])
                if li == 0:
                    add_dep_helper(wsl.ins, src_dep.ins,
                                   reason="self after tbl0")
                elif k in prev_hw_byblock:
                    add_dep_helper(wsl.ins, prev_hw_byblock[k].ins,
                                   reason="self rows after h write")
                asl = sb.tile([P, 4], BF16, tag="asl")
                nc.vector.tensor_tensor(
                    out=asl[:, 0:H], in0=hb[:, es0:es0 + H],
                    in1=hb[:, es0 + H:es0 + 2 * H], op=mybir.AluOpType.add)
                aslk = sb.tile([P, 4], BF16, tag="aslk")
                nc.vector.scalar_tensor_tensor(
                    out=aslk[:, 0:H], in0=asl[:, 0:H], scalar=NEG,
                    op0=mybir.AluOpType.mult, in1=asl[:, 0:H],
                    op1=mybir.AluOpType.max)
                easl = sb.tile([P, 4], BF16, tag="easl")
                nc.scalar.activation(out=easl[:, 0:H], in_=aslk[:, 0:H],
                                     func=mybir.ActivationFunctionType.Exp)

                sidx = Lsb + b
                nc.vector.tensor_tensor(
                    out=rhs[:, sidx, 0:OC].rearrange("p (h o) -> p h o", o=O),
                    in0=hb[:, 0:OC].rearrange("p (h o) -> p h o", o=O),
                    in1=easl[:, 0:H].unsqueeze(2).to_broadcast([P, H, O]),
                    op=mybir.AluOpType.mult)
                nc.vector.tensor_copy(out=rhs[:, sidx, OC:RC],
                                      in_=easl[:, 0:H])

                u_ps = ps_u.tile([P, RC], F32, space="PSUM", tag="u")
                ranges = []
                if slo:
                    p0 = blk_lo[k] - soff
                    ranges.append((p0, p0 + slo))
                if shi:
                    p0 = blk_hi[k] - soff
                    ranges.append((p0, p0 + shi))
                first = True
                for (r0, r1) in ranges:
                    for j in range(r0, r1):
                        nc.tensor.matmul(
                            out=u_ps[:], lhsT=m[:, j * P:(j + 1) * P],
                            rhs=rhs[:, j, :], start=first, stop=False)
                        first = False
                nc.tensor.matmul(out=u_ps[:], lhsT=c_identb[:],
                                 rhs=rhs[:, sidx, :], start=first, stop=True)

                rec = sb.tile([P, 4], F32, tag="rec")
                nc.vector.reciprocal(out=rec[:, 0:H], in_=u_ps[:, OC:RC])
                obb = sb.tile([P, 256], F32, tag="obb")
                nc.vector.tensor_tensor(
                    out=obb[:, 0:OC].rearrange("p (h o) -> p h o", o=O),
                    in0=u_ps[:, 0:OC].rearrange("p (h o) -> p h o", o=O),
                    in1=rec[:, 0:H].unsqueeze(2).to_broadcast([P, H, O]),
                    op=mybir.AluOpType.mult)
                nc.vector.tensor_tensor(out=obb[:, 0:OC], in0=obb[:, 0:OC],
                                        in1=c_b[li][:],
                                        op=mybir.AluOpType.add)

                if last:
                    if ostage is None:
                        ostage = sbs.tile([P, B_O, 2], F32, tag="ostage")
                        ostage_blocks = []
                    nc.scalar.activation(
                        out=ostage[:nk, k % B_O, :], in_=obb[:nk, 0:2],
                        func=mybir.ActivationFunctionType.Relu)
                    ostage_blocks.append((k % B_O, k, nk))
                    if len(ostage_blocks) == B_O or k == NBLK - 1:
                        full = [(j, kk) for (j, kk, nn) in ostage_blocks
                                if nn == P]
                        part = [(j, kk, nn) for (j, kk, nn) in ostage_blocks
                                if nn != P]
                        if full:
                            j0, k0 = full[0]
                            nc.sync.dma_start(
                                out=out_own[k0 * P:(k0 + len(full)) * P, :]
                                    .rearrange("(a p) c -> p a c", p=P),
                                in_=ostage[:, j0:j0 + len(full), :])
                        for (j, kk, nn) in part:
                            nc.sync.dma_start(
                                out=out_own[kk * P:kk * P + nn, :],
                                in_=ostage[:nn, j, :])
                        ostage = None
                else:
                    orl = sb.tile([P, 256], F32, tag="orl")
                    nc.scalar.activation(
                        out=orl[:], in_=obb[:, 0:OC],
                        func=mybir.ActivationFunctionType.Relu)
                    li2 = li + 1
                    cols2 = USED[li2]
                    h2_ps = ps.tile([P, max(cols2, 8)], F32, space="PSUM",
                                    tag="h2ps")
                    nf = OC // P
                    for f in range(nf):
                        tp_ps = ps.tile([P, P], F32, space="PSUM", tag="tp")
                        nc.tensor.transpose(
                            out=tp_ps[:], in_=orl[:, f * P:(f + 1) * P],
                            identity=c_identf[:])
                        xt = sb.tile([P, P], F32R, tag=f"xt{f}")
                        nc.scalar.copy(out=xt[:], in_=tp_ps[:])
                        nc.tensor.matmul(
                            out=h2_ps[:, 0:cols2], lhsT=xt[:],
                            rhs=wprime[li2][f][:, 0:cols2],
                            start=(f == 0), stop=(f == nf - 1))
                    if hstage is None:
                        hstage = sbs.tile([P, B_H * USED[1]], BF16,
                                          tag="hstage")
                        hstage_blocks = []
                    hsv = hstage[:].rearrange("p (s c) -> p s c", c=cols2)
                    nc.scalar.copy(out=hsv[:nk, k % B_H, :],
                                   in_=h2_ps[:nk, 0:cols2])
                    hstage_blocks.append((k % B_H, k, nk))
                    if len(hstage_blocks) == B_H or k == NBLK - 1:
                        ws = flush_hown(hsv, hstage_blocks, li2)
                        for w in ws:
                            h_writes.append(w)
                            for (_, kk, _) in hstage_blocks:
                                hw_byblock[kk] = w
                        hstage = None


# --------------------------------------------------------------------------
# entry point
# --------------------------------------------------------------------------

_cache = {}
TRACE = False
last_result = None


def _plan_key(plans):
    return (plans["A"]["S_lo"], plans["A"]["S_hi"],
            plans["B"]["S_lo"], plans["B"]["S_hi"])


def kernel(x, edge_index, W0, a_src0, a_dst0, b0, W1, a_src1, a_dst1, b1,
           W2, a_src2, a_dst2, b2):
    weights = [(W0, a_src0, a_dst0, b0), (W1, a_src1, a_dst1, b1),
               (W2, a_src2, a_dst2, b2)]
    in_maps, plans = _host_prep(np.asarray(x), np.asarray(edge_index), weights)

    key = _plan_key(plans)
    if key not in _cache:
        _cache[key] = build_program(plans)
    nc = _cache[key]

    global last_result
    res = run_bass_kernel_spmd(nc, in_maps, core_ids=list(range(GEO.NCORES)),
                               trace=TRACE)
    last_result = res
    out = np.concatenate(
        [res.results[d]["out"] for d in range(GEO.NCORES)], axis=0)
    return out.astype(np.float32)
